# revision 20
# baseline (speedup 1.0000x reference)
"""Trainium2 Bass kernel for masked edge pooling + linear (nn_EtoX).

Reference computation (per sample b, node i, over neighbors j with mask[b, j]):
  m   = sum_j E[b,i,j,:] / count_b          (unmasked sum / masked count)
  mi  = min over present j of E[b,i,j,:]
  ma  = max over present j of E[b,i,j,:]
  std = sum_{present j} (E - m)^2 / count_b
  out = concat(m, mi, ma, std) @ W.T + bias

Strategy v2: data-parallel over batch (2 samples per core, 8 cores). The host
permutes each sample's j axis present-first (pads duplicate the first present
row) and appends the absent rows (padded to CApad with duplicates), then casts
to fp16. One contiguous DMA per 128-row i-block brings the merged
[128, 256+CApad, 64] fp16 slab in; all reductions are pairwise fp16
tensor_tensor trees on VectorE at 2x rate:
  - min/max trees over the 256 present-padded rows (pads are neutral)
  - sum tree (pad contribution subtracted exactly via npadP * x0)
  - ScalarE squares the slab; a second tree gives the present sum of squares
  - GpSimd reduces the absent block for the mean's unmasked-sum correction
The epilogue forms m and std in fp32 ([P,64] tiles, no parity split), packs
z = [m|mi] / [ma|std], and TensorE transposes + applies the 256x256 linear.
"""

import os

# Whole-tile dependency granularity: lets a 1-element ACT "fence" write
# supersede a DMA-landed tile's reader/writer dep set, keeping every DMA
# instruction within the hardware's 2-sync-wait budget.
os.environ.setdefault("BY_DEFAULT_DISABLE_SUBTILE_DEPS", "1")

import numpy as np

try:
    from concourse import bass, mybir, tile
    from concourse.bass_utils import run_bass_kernel_spmd
except ImportError:  # fall back to the container's repo checkout
    import sys

    sys.path.insert(0, "/opt/trn_rl_repo")
    from concourse import bass, mybir, tile
    from concourse.bass_utils import run_bass_kernel_spmd

BS, N, DE, DX = 16, 256, 64, 256
FI = 4 * DE
NCORES = 8
BPC = BS // NCORES  # samples per core
P = 128

F32 = mybir.dt.float32
F16 = mybir.dt.float16

LAST_RESULT = {}

_NC_CACHE = {}


def _enable_tracing():
    """Install the NTFF profile hook that the image's ``antenv`` lacks."""
    import contextlib
    import ctypes
    import sys
    import types

    try:
        import antenv.axon_hooks  # noqa: F401

        pass
    except ImportError:
        so_path = "/opt/axon/libaxon_pjrt.so"
        lib = ctypes.CDLL(so_path)
        if hasattr(lib, "axon_start_nrt_profile"):
            lib.axon_start_nrt_profile.argtypes = [
                ctypes.POINTER(ctypes.c_int64),
                ctypes.c_size_t,
            ]
            lib.axon_start_nrt_profile.restype = ctypes.c_int64
            lib.axon_stop_nrt_profile.argtypes = [ctypes.c_char_p]
            lib.axon_stop_nrt_profile.restype = ctypes.c_int64

            @contextlib.contextmanager
            def _hook(output_dir, device_ids):
                import jax

                jax.devices()
                if device_ids:
                    ids = (ctypes.c_int64 * len(device_ids))(*device_ids)
                    rc = lib.axon_start_nrt_profile(ids, len(device_ids))
                else:
                    rc = lib.axon_start_nrt_profile(None, 0)
                if rc != 0:
                    raise RuntimeError(f"axon_start_nrt_profile rc={rc}")
                try:
                    yield
                finally:
                    n = lib.axon_stop_nrt_profile(str(output_dir).encode())
                    print(f"profile: {n} file(s) written to {output_dir}")

            mod = types.ModuleType("antenv.axon_hooks")
            mod.get_axon_ntff_profile_hook = lambda: _hook
            mod.set_axon_ntff_profile_hook = lambda h: None
            import antenv

            sys.modules["antenv.axon_hooks"] = mod
            antenv.axon_hooks = mod

    from concourse import bass_utils as _bu

    _bu.upload_artifacts = lambda tmpdir: f"file://{tmpdir}"


def _hoist_excess_waits(bir: dict) -> dict:
    """Walrus (this build) rejects instructions whose embedded sync-wait list
    exceeds the ISA struct's slots. Hoist all but one wait into standalone
    single-wait EventSemaphore instructions placed immediately before the
    instruction on the same engine stream - semantically identical."""
    ctr = 0
    for fn in bir["functions"]:
        for blk in fn["blocks"]:
            new = []
            for ins in blk["instructions"]:
                si = ins.get("sync_info")
                if si:
                    waits = si.get("on_wait") or []
                    if len(waits) > 1:
                        for w in waits[:-1]:
                            ctr += 1
                            new.append(
                                {
                                    "debug": ins.get("debug", 0),
                                    "engine": ins["engine"],
                                    "ins": [],
                                    "outs": [],
                                    "name": f"hoistw-{ctr}",
                                    "opcode": "EventSemaphore",
                                    "sync_info": {"on_update": [], "on_wait": [w]},
                                }
                            )
                        si["on_wait"] = [waits[-1]]
                new.append(ins)
            blk["instructions"] = new
    return bir


def build_program(CApad: int) -> "bass.Bass":
    nc = bass.Bass()
    NI = BPC * N  # flattened (sample, i) rows
    W_ROW = N + CApad  # merged row length in j
    eg = nc.declare_dram_parameter("eg", [NI, W_ROW, DE], F16, isOutput=False)
    wt = nc.declare_dram_parameter("wt", [FI, DX], F32, isOutput=False)
    brow = nc.declare_dram_parameter("brow", [1, DX], F32, isOutput=False)
    ident = nc.declare_dram_parameter("ident", [P, P], F32, isOutput=False)
    scal = nc.declare_dram_parameter("scal", [BPC, P, 3], F32, isOutput=False)
    out = nc.declare_dram_parameter("out", [NI, DX], F32, isOutput=True)

    MIN = mybir.AluOpType.min
    MAX = mybir.AluOpType.max
    ADD = mybir.AluOpType.add
    SUB = mybir.AluOpType.subtract
    MUL = mybir.AluOpType.mult

    # SDMA-CCE accumulate DMAs crash this runtime (JaxRuntimeError INTERNAL on
    # both HBM->SBUF and SBUF->SBUF accum_op paths) - keep disabled.
    USE_CCE_S = os.environ.get("NN_CCE_S", "0") == "1"
    USE_CCE_Q = os.environ.get("NN_CCE_Q", "0") == "1"

    with tile.TileContext(nc) as tc:
        with (
            tc.tile_pool(name="singles", bufs=1) as singles,
            tc.tile_pool(name="main", bufs=2) as main,
            tc.tile_pool(name="sq", bufs=1) as sqp,
            tc.tile_pool(name="trees", bufs=1) as trees,
            tc.tile_pool(name="stats", bufs=2) as stats,
            tc.tile_pool(name="ep", bufs=1) as ep,
            tc.tile_pool(name="outp", bufs=2) as outp,
            tc.tile_pool(name="psum", bufs=2, space="PSUM") as psum,
        ):
            wt0 = singles.tile([P, DX], F32, tag="wt0")
            nc.sync.dma_start(out=wt0[:], in_=wt[0:P, :])
            wt1 = singles.tile([P, DX], F32, tag="wt1")
            nc.sync.dma_start(out=wt1[:], in_=wt[P:FI, :])
            id_t = singles.tile([P, P], F32, tag="id")
            nc.sync.dma_start(out=id_t[:], in_=ident[:, :])
            br_t = singles.tile([1, DX], F32, tag="br")
            nc.sync.dma_start(out=br_t[:], in_=brow[:, :])
            ones1 = singles.tile([1, P], F32, tag="ones")
            nc.vector.memset(ones1[:], 1.0)
            sc = {}
            for b in range(BPC):
                for k, nm in enumerate(("npadP", "npadA", "invCP")):
                    t = singles.tile([P, 1], F32, tag=f"sc{b}{k}")
                    nc.sync.dma_start(out=t[:], in_=scal[b, :, k : k + 1])
                    sc[(b, nm)] = t

            # shared DVE tree scratch (DVE-serial, bufs=1 is fine)
            tA = trees.tile([P, P, DE], F16, tag="treeA")
            tB = trees.tile([P, 64, DE], F16, tag="treeB")

            def tree_down(op, src, w0, dst_f32):
                """Pairwise-reduce src[:, 0:2*w0, :] (fp16) over j into the
                fp32 [P, 64] AP dst_f32, ping-ponging through tB/tA."""
                cur, nxt = src, tB
                w = w0
                while w >= 2:
                    nc.vector.tensor_tensor(
                        nxt[:, 0:w, :], cur[:, 0:w, :], cur[:, w : 2 * w, :], op
                    )
                    cur = nxt
                    nxt = tA if nxt is tB else tB
                    w //= 2
                nc.vector.tensor_tensor(
                    dst_f32,
                    cur[:, 0:1, :].rearrange("p a d -> p (a d)"),
                    cur[:, 1:2, :].rearrange("p a d -> p (a d)"),
                    op,
                )

            # packed-tail staging: [S-L3 | Q-L3 | abs-L1] as 3 groups of 32 rows
            pk0 = trees.tile([P, 3 * 32, DE], F16, tag="pk0")
            pk1 = trees.tile([P, 3 * 16, DE], F16, tag="pk1")

            for b in range(BPC):
                # per-sample stat tiles: index 'a' is the i-half (ih)
                zS01 = stats.tile([P, 2, P], F32, tag="z01")  # per ih: [m | mi]
                zS23 = stats.tile([P, 2, P], F32, tag="z23")  # per ih: [ma | std]
                SQA = stats.tile([P, 2, 3, DE], F32, tag="SQA")  # (S|Q|Sa) pad sums
                x0f = stats.tile([P, 2, DE], F32, tag="x0f")
                xaf = stats.tile([P, 2, DE], F32, tag="xaf")

                for ih in range(2):
                    r0 = b * N + ih * P  # row offset in eg/out
                    mt0 = main.tile([P, P, DE], F16, tag="mt0")
                    nc.sync.dma_start(out=mt0[:], in_=eg[r0 : r0 + P, 0:P, :])
                    mt1 = main.tile([P, P, DE], F16, tag="mt1")
                    nc.sync.dma_start(out=mt1[:], in_=eg[r0 : r0 + P, P:N, :])
                    mta = main.tile([P, CApad, DE], F16, tag="mta")
                    nc.sync.dma_start(out=mta[:], in_=eg[r0 : r0 + P, N : N + CApad, :])

                    if USE_CCE_S:
                        # S-tree L1 on the SDMA CCE: re-read both halves from
                        # HBM, second with accumulate -> tS = h0 + h1
                        tS = sqp.tile([P, P, DE], F16, tag="tS")
                        nc.sync.dma_start(out=tS[:], in_=eg[r0 : r0 + P, 0:P, :])
                        nc.gpsimd.dma_start(
                            out=tS[:],
                            in_=eg[r0 : r0 + P, P:N, :],
                            accum_op=ADD,
                        )

                    # ScalarE: squares (for sumsq tree) + fp32 dup-row copies
                    sq0 = sqp.tile([P, P, DE], F16, tag="sq0")
                    nc.scalar.activation(
                        out=sq0[:], in_=mt0[:], func=mybir.ActivationFunctionType.Square
                    )
                    sq1 = sqp.tile([P, P, DE], F16, tag="sq1")
                    nc.scalar.activation(
                        out=sq1[:], in_=mt1[:], func=mybir.ActivationFunctionType.Square
                    )
                    if USE_CCE_Q:
                        # Q-tree L1 on the SDMA CCE: sq0 += sq1 (SBUF->SBUF)
                        nc.gpsimd.dma_start(out=sq0[:], in_=sq1[:], accum_op=ADD)
                    nc.scalar.copy(out=x0f[:, ih, :], in_=mt0[:, 0, :])
                    nc.scalar.copy(out=xaf[:, ih, :], in_=mta[:, 0, :])

                    # DVE order matters (emission order = execution order):
                    # min/max first (only need mt0/mt1, which land earliest),
                    # S next, Q last so ScalarE squares have a full block of
                    # slack; the shared ADD tail then folds S/abs/Q together.
                    TT = nc.vector.tensor_tensor
                    TT(tA[:, 0:64, :], mt0[:, 0:64, :], mt0[:, 64:P, :], MIN)
                    TT(tA[:, 64:P, :], mt1[:, 0:64, :], mt1[:, 64:P, :], MIN)
                    tree_down(MIN, tA, 64, zS01[:, ih, 64:128])
                    TT(tA[:, 0:64, :], mt0[:, 0:64, :], mt0[:, 64:P, :], MAX)
                    TT(tA[:, 64:P, :], mt1[:, 0:64, :], mt1[:, 64:P, :], MAX)
                    tree_down(MAX, tA, 64, zS23[:, ih, 0:64])

                    if USE_CCE_S:
                        TT(tB[:, 0:64, :], tS[:, 0:64, :], tS[:, 64:P, :], ADD)
                    else:
                        TT(tA[:, 0:64, :], mt0[:, 0:64, :], mt0[:, 64:P, :], ADD)
                        TT(tA[:, 64:P, :], mt1[:, 0:64, :], mt1[:, 64:P, :], ADD)
                        TT(tB[:, 0:64, :], tA[:, 0:64, :], tA[:, 64:P, :], ADD)
                    TT(pk0[:, 0:32, :], tB[:, 0:32, :], tB[:, 32:64, :], ADD)
                    if CApad == 64:
                        TT(pk0[:, 64:96, :], mta[:, 0:32, :], mta[:, 32:64, :], ADD)
                    else:  # CApad == 128: one extra pre-level
                        TT(tB[:, 0:64, :], mta[:, 0:64, :], mta[:, 64:P, :], ADD)
                        TT(pk0[:, 64:96, :], tB[:, 0:32, :], tB[:, 32:64, :], ADD)
                    if USE_CCE_Q:
                        TT(tB[:, 0:64, :], sq0[:, 0:64, :], sq0[:, 64:P, :], ADD)
                    else:
                        TT(tA[:, 0:64, :], sq0[:, 0:64, :], sq0[:, 64:P, :], ADD)
                        TT(tA[:, 64:P, :], sq1[:, 0:64, :], sq1[:, 64:P, :], ADD)
                        TT(tB[:, 0:64, :], tA[:, 0:64, :], tA[:, 64:P, :], ADD)
                    TT(pk0[:, 32:64, :], tB[:, 0:32, :], tB[:, 32:64, :], ADD)

                    v32 = pk0[:, 0:96, :].rearrange("p (g w) d -> p g w d", g=3)
                    v16 = pk1[:, 0:48, :].rearrange("p (g w) d -> p g w d", g=3)
                    TT(v16, v32[:, :, 0:16, :], v32[:, :, 16:32, :], ADD)
                    v8 = pk0[:, 0:24, :].rearrange("p (g w) d -> p g w d", g=3)
                    TT(v8, v16[:, :, 0:8, :], v16[:, :, 8:16, :], ADD)
                    v4 = pk1[:, 0:12, :].rearrange("p (g w) d -> p g w d", g=3)
                    TT(v4, v8[:, :, 0:4, :], v8[:, :, 4:8, :], ADD)
                    v2 = pk0[:, 0:6, :].rearrange("p (g w) d -> p g w d", g=3)
                    TT(v2, v4[:, :, 0:2, :], v4[:, :, 2:4, :], ADD)
                    TT(SQA[:, ih, :, :], v2[:, :, 0, :], v2[:, :, 1, :], ADD)

                    # fences: collapse reader sets before buffer reuse
                    nc.scalar.mul(mt0[0:1, 0:1, 0:1], mt0[0:1, 0:1, 0:1], 0.0)
                    nc.scalar.mul(mt1[0:1, 0:1, 0:1], mt1[0:1, 0:1, 0:1], 0.0)
                    nc.scalar.mul(mta[0:1, 0:1, 0:1], mta[0:1, 0:1, 0:1], 0.0)
                    if USE_CCE_S:
                        nc.scalar.mul(tS[0:1, 0:1, 0:1], tS[0:1, 0:1, 0:1], 0.0)
                    nc.scalar.mul(sq0[0:1, 0:1, 0:1], sq0[0:1, 0:1, 0:1], 0.0)
                    nc.scalar.mul(sq1[0:1, 0:1, 0:1], sq1[0:1, 0:1, 0:1], 0.0)

                # per-sample epilogue: [P,2,64] APs, both i-halves at once.
                # Per-partition-scalar multiplies ride ScalarE (activation
                # scale); the tensor+tensor ops stay on DVE.
                Sp_v = SQA[:, :, 0, :]
                Qp_v = SQA[:, :, 1, :]
                Sa_v = SQA[:, :, 2, :]

                def et(tag):
                    return ep.tile([P, 2, DE], F32, tag=tag, name=tag)

                TT = nc.vector.tensor_tensor
                tP_ = et("tP")
                nc.scalar.mul(tP_[:], x0f[:], sc[(b, "npadP")][:])
                tA2 = et("tA2")
                nc.scalar.mul(tA2[:], xaf[:], sc[(b, "npadA")][:])
                Spres = et("Spres")
                TT(Spres[:], Sp_v, tP_[:], SUB)
                Sabs = et("Sabs")
                TT(Sabs[:], Sa_v, tA2[:], SUB)
                tQ_ = et("tQ")
                TT(tQ_[:], tP_[:], x0f[:], MUL)
                Qpres = et("Qpres")
                TT(Qpres[:], Qp_v, tQ_[:], SUB)
                sall = et("sall")
                TT(sall[:], Spres[:], Sabs[:], ADD)
                m_dst = zS01[:, :, 0:64]  # strided 3D AP
                nc.scalar.mul(m_dst, sall[:], sc[(b, "invCP")][:])
                d_t = et("d")
                TT(d_t[:], Spres[:], Sabs[:], SUB)
                e_t = et("e")
                TT(e_t[:], m_dst, d_t[:], MUL)
                f_t = et("f")
                TT(f_t[:], Qpres[:], e_t[:], SUB)
                nc.scalar.mul(zS23[:, :, 64:128], f_t[:], sc[(b, "invCP")][:])

                # transpose packed stats into z^T layout ([feature, i]) + linear
                for ih in range(2):
                    r0 = b * N + ih * P
                    psz0 = psum.tile([P, P], F32, tag="psz0")
                    nc.tensor.transpose(out=psz0[:], in_=zS01[:, ih, :], identity=id_t[:])
                    psz1 = psum.tile([P, P], F32, tag="psz1")
                    nc.tensor.transpose(out=psz1[:], in_=zS23[:, ih, :], identity=id_t[:])
                    zT0 = outp.tile([P, P], F32, tag="zT0")
                    nc.scalar.copy(out=zT0[:], in_=psz0[:])
                    zT1 = outp.tile([P, P], F32, tag="zT1")
                    nc.scalar.copy(out=zT1[:], in_=psz1[:])

                    pso = psum.tile([P, DX], F32, tag="pso")
                    nc.tensor.matmul(pso[:], zT0[:], wt0[:], start=True, stop=False)
                    nc.tensor.matmul(pso[:], zT1[:], wt1[:], start=False, stop=False)
                    nc.tensor.matmul(pso[:], ones1[:], br_t[:], start=False, stop=True)
                    o_t = outp.tile([P, DX], F32, tag="o_t")
                    nc.scalar.copy(out=o_t[:], in_=pso[:])
                    nc.scalar.dma_start(out=out[r0 : r0 + P, :], in_=o_t[:])

    import json as _json

    _orig_to_json = nc.to_json_bytes

    def _patched_to_json():
        return _json.dumps(_hoist_excess_waits(_json.loads(_orig_to_json()))).encode()

    nc.to_json_bytes = _patched_to_json
    return nc


def kernel(E, e_mask2, W, b):
    E = np.asarray(E, dtype=np.float32)
    mask = np.asarray(e_mask2).reshape(BS, N).astype(bool)
    W = np.asarray(W, dtype=np.float32)
    bv = np.asarray(b, dtype=np.float32)

    pj = [np.nonzero(mask[s])[0] for s in range(BS)]
    aj = [np.nonzero(~mask[s])[0] for s in range(BS)]
    cPs = [len(x) for x in pj]
    cAs = [len(x) for x in aj]
    assert all(c > 0 for c in cPs), "a sample with zero present edges divides by zero"
    CA = max(1, max(cAs))
    CApad = 64 if CA <= 64 else 128
    assert CA <= 128

    perms = []
    for s in range(BS):
        pad_p = np.full(N - cPs[s], pj[s][0], dtype=np.int64)
        if cAs[s] > 0:
            tail = np.concatenate(
                [aj[s], np.full(CApad - cAs[s], aj[s][0], dtype=np.int64)]
            )
        else:
            tail = np.full(CApad, pj[s][0], dtype=np.int64)
        perms.append(np.concatenate([pj[s], pad_p, tail]))

    WT = np.ascontiguousarray(W.T)  # [FI, DX]
    ident = np.eye(P, dtype=np.float32)
    brow = np.ascontiguousarray(bv.reshape(1, DX))

    if CApad not in _NC_CACHE:
        _NC_CACHE[CApad] = build_program(CApad)
    nc = _NC_CACHE[CApad]

    in_maps = []
    for c in range(NCORES):
        egs = np.empty((BPC * N, N + CApad, DE), np.float16)
        scals = np.empty((BPC, P, 3), np.float32)
        for bl in range(BPC):
            s = c * BPC + bl
            egs[bl * N : (bl + 1) * N] = E[s][:, perms[s], :].astype(np.float16)
            npadA = (CApad - cAs[s]) if cAs[s] > 0 else CApad
            scals[bl, :] = (float(N - cPs[s]), float(npadA), 1.0 / cPs[s])
        in_maps.append(
            {"eg": egs, "wt": WT, "brow": brow, "ident": ident, "scal": scals}
        )

    trace = os.environ.get("NN_KERNEL_TRACE", "0") == "1"
    if trace:
        _enable_tracing()
    res = run_bass_kernel_spmd(
        nc, in_maps, list(range(NCORES)), trace=trace, tmpdir="/tmp/nn_kernel_trace"
    )
    LAST_RESULT["exec_time_ns"] = res.exec_time_ns
    LAST_RESULT["mean_exec_time_ns"] = res.mean_exec_time_ns
    LAST_RESULT["profile_json"] = res.profile_json

    out = np.concatenate(
        [res.results[c]["out"].reshape(BPC, N, DX) for c in range(NCORES)], axis=0
    )
    return out.astype(np.float32)


# revision 24
# speedup vs baseline: 1.1377x; 1.1377x over previous
"""Trainium2 Bass kernel for masked edge pooling + linear (nn_EtoX).

Reference computation (per sample b, node i, over neighbors j with mask[b, j]):
  m   = sum_j E[b,i,j,:] / count_b          (unmasked sum / masked count)
  mi  = min over present j of E[b,i,j,:]
  ma  = max over present j of E[b,i,j,:]
  std = sum_{present j} (E - m)^2 / count_b
  out = concat(m, mi, ma, std) @ W.T + bias

Strategy v2: data-parallel over batch (2 samples per core, 8 cores). The host
permutes each sample's j axis present-first (pads duplicate the first present
row) and appends the absent rows (padded to CApad with duplicates), then casts
to fp16. One contiguous DMA per 128-row i-block brings the merged
[128, 256+CApad, 64] fp16 slab in; all reductions are pairwise fp16
tensor_tensor trees on VectorE at 2x rate:
  - min/max trees over the 256 present-padded rows (pads are neutral)
  - sum tree (pad contribution subtracted exactly via npadP * x0)
  - ScalarE squares the slab; a second tree gives the present sum of squares
  - GpSimd reduces the absent block for the mean's unmasked-sum correction
The epilogue forms m and std in fp32 ([P,64] tiles, no parity split), packs
z = [m|mi] / [ma|std], and TensorE transposes + applies the 256x256 linear.
"""

import os

# Whole-tile dependency granularity: lets a 1-element ACT "fence" write
# supersede a DMA-landed tile's reader/writer dep set, keeping every DMA
# instruction within the hardware's 2-sync-wait budget.
os.environ.setdefault("BY_DEFAULT_DISABLE_SUBTILE_DEPS", "1")

import numpy as np

try:
    from concourse import bass, mybir, tile
    from concourse.bass_utils import run_bass_kernel_spmd
except ImportError:  # fall back to the container's repo checkout
    import sys

    sys.path.insert(0, "/opt/trn_rl_repo")
    from concourse import bass, mybir, tile
    from concourse.bass_utils import run_bass_kernel_spmd

BS, N, DE, DX = 16, 256, 64, 256
FI = 4 * DE
NCORES = 8
BPC = BS // NCORES  # samples per core
P = 128

F32 = mybir.dt.float32
F16 = mybir.dt.float16

LAST_RESULT = {}

_NC_CACHE = {}


def _enable_tracing():
    """Install the NTFF profile hook that the image's ``antenv`` lacks."""
    import contextlib
    import ctypes
    import sys
    import types

    try:
        import antenv.axon_hooks  # noqa: F401

        pass
    except ImportError:
        so_path = "/opt/axon/libaxon_pjrt.so"
        lib = ctypes.CDLL(so_path)
        if hasattr(lib, "axon_start_nrt_profile"):
            lib.axon_start_nrt_profile.argtypes = [
                ctypes.POINTER(ctypes.c_int64),
                ctypes.c_size_t,
            ]
            lib.axon_start_nrt_profile.restype = ctypes.c_int64
            lib.axon_stop_nrt_profile.argtypes = [ctypes.c_char_p]
            lib.axon_stop_nrt_profile.restype = ctypes.c_int64

            @contextlib.contextmanager
            def _hook(output_dir, device_ids):
                import jax

                jax.devices()
                if device_ids:
                    ids = (ctypes.c_int64 * len(device_ids))(*device_ids)
                    rc = lib.axon_start_nrt_profile(ids, len(device_ids))
                else:
                    rc = lib.axon_start_nrt_profile(None, 0)
                if rc != 0:
                    raise RuntimeError(f"axon_start_nrt_profile rc={rc}")
                try:
                    yield
                finally:
                    n = lib.axon_stop_nrt_profile(str(output_dir).encode())
                    print(f"profile: {n} file(s) written to {output_dir}")

            mod = types.ModuleType("antenv.axon_hooks")
            mod.get_axon_ntff_profile_hook = lambda: _hook
            mod.set_axon_ntff_profile_hook = lambda h: None
            import antenv

            sys.modules["antenv.axon_hooks"] = mod
            antenv.axon_hooks = mod

    from concourse import bass_utils as _bu

    _bu.upload_artifacts = lambda tmpdir: f"file://{tmpdir}"


def _hoist_excess_waits(bir: dict) -> dict:
    """Walrus (this build) rejects instructions whose embedded sync-wait list
    exceeds the ISA struct's slots. Hoist all but one wait into standalone
    single-wait EventSemaphore instructions placed immediately before the
    instruction on the same engine stream - semantically identical."""
    ctr = 0
    for fn in bir["functions"]:
        for blk in fn["blocks"]:
            new = []
            for ins in blk["instructions"]:
                si = ins.get("sync_info")
                if si:
                    waits = si.get("on_wait") or []
                    if len(waits) > 1:
                        for w in waits[:-1]:
                            ctr += 1
                            new.append(
                                {
                                    "debug": ins.get("debug", 0),
                                    "engine": ins["engine"],
                                    "ins": [],
                                    "outs": [],
                                    "name": f"hoistw-{ctr}",
                                    "opcode": "EventSemaphore",
                                    "sync_info": {"on_update": [], "on_wait": [w]},
                                }
                            )
                        si["on_wait"] = [waits[-1]]
                new.append(ins)
            blk["instructions"] = new
    return bir


def build_program(CApad: int) -> "bass.Bass":
    nc = bass.Bass()
    NI = BPC * N  # flattened (sample, i) rows
    W_ROW = N + CApad  # merged row length in j
    eg = nc.declare_dram_parameter("eg", [NI, W_ROW, DE], F16, isOutput=False)
    wt = nc.declare_dram_parameter("wt", [FI, DX], F32, isOutput=False)
    brow = nc.declare_dram_parameter("brow", [1, DX], F32, isOutput=False)
    ident = nc.declare_dram_parameter("ident", [P, P], F32, isOutput=False)
    scal = nc.declare_dram_parameter("scal", [BPC, P, 3], F32, isOutput=False)
    out = nc.declare_dram_parameter("out", [NI, DX], F32, isOutput=True)

    MIN = mybir.AluOpType.min
    MAX = mybir.AluOpType.max
    ADD = mybir.AluOpType.add
    SUB = mybir.AluOpType.subtract
    MUL = mybir.AluOpType.mult

    # SDMA-CCE accumulate DMAs crash this runtime (JaxRuntimeError INTERNAL on
    # both HBM->SBUF and SBUF->SBUF accum_op paths) - keep disabled.
    USE_CCE_S = os.environ.get("NN_CCE_S", "0") == "1"
    USE_CCE_Q = os.environ.get("NN_CCE_Q", "0") == "1"

    with tile.TileContext(nc) as tc:
        with (
            tc.tile_pool(name="singles", bufs=1) as singles,
            tc.tile_pool(name="main", bufs=2) as main,
            tc.tile_pool(name="sq", bufs=1) as sqp,
            tc.tile_pool(name="trees", bufs=1) as trees,
            tc.tile_pool(name="stats", bufs=2) as stats,
            tc.tile_pool(name="ep", bufs=1) as ep,
            tc.tile_pool(name="outp", bufs=2) as outp,
            tc.tile_pool(name="psum", bufs=2, space="PSUM") as psum,
        ):
            # singles ride the scalar HWDGE ring so the sync ring's FIFO
            # starts with the first data tile immediately
            wt0 = singles.tile([P, DX], F32, tag="wt0")
            nc.scalar.dma_start(out=wt0[:], in_=wt[0:P, :])
            wt1 = singles.tile([P, DX], F32, tag="wt1")
            nc.scalar.dma_start(out=wt1[:], in_=wt[P:FI, :])
            id_t = singles.tile([P, P], F32, tag="id")
            nc.scalar.dma_start(out=id_t[:], in_=ident[:, :])
            br_t = singles.tile([1, DX], F32, tag="br")
            nc.scalar.dma_start(out=br_t[:], in_=brow[:, :])
            ones1 = singles.tile([1, P], F32, tag="ones")
            nc.vector.memset(ones1[:], 1.0)
            sc = {}
            for b in range(BPC):
                for k, nm in enumerate(("npadP", "npadA", "invCP")):
                    t = singles.tile([P, 1], F32, tag=f"sc{b}{k}")
                    nc.scalar.dma_start(out=t[:], in_=scal[b, :, k : k + 1])
                    sc[(b, nm)] = t

            # shared DVE tree scratch (DVE-serial, bufs=1 is fine)
            tA = trees.tile([P, P, DE], F16, tag="treeA")
            tB = trees.tile([P, 64, DE], F16, tag="treeB")

            def tree_down(op, src, w0, dst_f32):
                """Pairwise-reduce src[:, 0:2*w0, :] (fp16) over j into the
                fp32 [P, 64] AP dst_f32, ping-ponging through tB/tA."""
                cur, nxt = src, tB
                w = w0
                while w >= 2:
                    nc.vector.tensor_tensor(
                        nxt[:, 0:w, :], cur[:, 0:w, :], cur[:, w : 2 * w, :], op
                    )
                    cur = nxt
                    nxt = tA if nxt is tB else tB
                    w //= 2
                nc.vector.tensor_tensor(
                    dst_f32,
                    cur[:, 0:1, :].rearrange("p a d -> p (a d)"),
                    cur[:, 1:2, :].rearrange("p a d -> p (a d)"),
                    op,
                )

            # packed-tail staging: [S-L3 | Q-L3 | abs-L1] as 3 groups of 32 rows
            pk0 = trees.tile([P, 3 * 32, DE], F16, tag="pk0")
            pk1 = trees.tile([P, 3 * 16, DE], F16, tag="pk1")

            for b in range(BPC):
                # per-sample stat tiles: index 'a' is the i-half (ih)
                zS01 = stats.tile([P, 2, P], F32, tag="z01")  # per ih: [m | mi]
                zS23 = stats.tile([P, 2, P], F32, tag="z23")  # per ih: [ma | std]
                SQA = stats.tile([P, 2, 3, DE], F32, tag="SQA")  # (S|Q|Sa) pad sums
                x0f = stats.tile([P, 2, DE], F32, tag="x0f")
                xaf = stats.tile([P, 2, DE], F32, tag="xaf")

                last = b == BPC - 1
                blk = {}

                def emit_load(ih):
                    r0 = b * N + ih * P
                    mt0 = main.tile([P, P, DE], F16, tag="mt0")
                    nc.sync.dma_start(out=mt0[:], in_=eg[r0 : r0 + P, 0:P, :])
                    mt1 = main.tile([P, P, DE], F16, tag="mt1")
                    nc.sync.dma_start(out=mt1[:], in_=eg[r0 : r0 + P, P:N, :])
                    mta = main.tile([P, CApad, DE], F16, tag="mta")
                    nc.sync.dma_start(out=mta[:], in_=eg[r0 : r0 + P, N : N + CApad, :])
                    sq0 = sqp.tile([P, P, DE], F16, tag="sq0")
                    nc.scalar.activation(
                        out=sq0[:], in_=mt0[:], func=mybir.ActivationFunctionType.Square
                    )
                    sq1 = sqp.tile([P, P, DE], F16, tag="sq1")
                    nc.scalar.activation(
                        out=sq1[:], in_=mt1[:], func=mybir.ActivationFunctionType.Square
                    )
                    nc.scalar.copy(out=x0f[:, ih, :], in_=mt0[:, 0, :])
                    nc.scalar.copy(out=xaf[:, ih, :], in_=mta[:, 0, :])
                    blk[ih] = (mt0, mt1, mta, sq0, sq1)

                TT = nc.vector.tensor_tensor

                def emit_minmax(ih):
                    mt0, mt1, mta, sq0, sq1 = blk[ih]
                    TT(tA[:, 0:64, :], mt0[:, 0:64, :], mt0[:, 64:P, :], MIN)
                    TT(tA[:, 64:P, :], mt1[:, 0:64, :], mt1[:, 64:P, :], MIN)
                    tree_down(MIN, tA, 64, zS01[:, ih, 64:128])
                    TT(tA[:, 0:64, :], mt0[:, 0:64, :], mt0[:, 64:P, :], MAX)
                    TT(tA[:, 64:P, :], mt1[:, 0:64, :], mt1[:, 64:P, :], MAX)
                    tree_down(MAX, tA, 64, zS23[:, ih, 0:64])

                def emit_sums(ih):
                    mt0, mt1, mta, sq0, sq1 = blk[ih]
                    TT(tA[:, 0:64, :], mt0[:, 0:64, :], mt0[:, 64:P, :], ADD)
                    TT(tA[:, 64:P, :], mt1[:, 0:64, :], mt1[:, 64:P, :], ADD)
                    TT(tB[:, 0:64, :], tA[:, 0:64, :], tA[:, 64:P, :], ADD)
                    TT(pk0[:, 0:32, :], tB[:, 0:32, :], tB[:, 32:64, :], ADD)
                    if CApad == 64:
                        TT(pk0[:, 64:96, :], mta[:, 0:32, :], mta[:, 32:64, :], ADD)
                    else:  # CApad == 128: one extra pre-level
                        TT(tB[:, 0:64, :], mta[:, 0:64, :], mta[:, 64:P, :], ADD)
                        TT(pk0[:, 64:96, :], tB[:, 0:32, :], tB[:, 32:64, :], ADD)
                    TT(tA[:, 0:64, :], sq0[:, 0:64, :], sq0[:, 64:P, :], ADD)
                    TT(tA[:, 64:P, :], sq1[:, 0:64, :], sq1[:, 64:P, :], ADD)
                    TT(tB[:, 0:64, :], tA[:, 0:64, :], tA[:, 64:P, :], ADD)
                    TT(pk0[:, 32:64, :], tB[:, 0:32, :], tB[:, 32:64, :], ADD)

                    v32 = pk0[:, 0:96, :].rearrange("p (g w) d -> p g w d", g=3)
                    v16 = pk1[:, 0:48, :].rearrange("p (g w) d -> p g w d", g=3)
                    TT(v16, v32[:, :, 0:16, :], v32[:, :, 16:32, :], ADD)
                    v8 = pk0[:, 0:24, :].rearrange("p (g w) d -> p g w d", g=3)
                    TT(v8, v16[:, :, 0:8, :], v16[:, :, 8:16, :], ADD)
                    v4 = pk1[:, 0:12, :].rearrange("p (g w) d -> p g w d", g=3)
                    TT(v4, v8[:, :, 0:4, :], v8[:, :, 4:8, :], ADD)
                    v2 = pk0[:, 0:6, :].rearrange("p (g w) d -> p g w d", g=3)
                    TT(v2, v4[:, :, 0:2, :], v4[:, :, 2:4, :], ADD)
                    TT(SQA[:, ih, :, :], v2[:, :, 0, :], v2[:, :, 1, :], ADD)

                def emit_fences(ih):
                    for t_ in blk[ih]:
                        nc.scalar.mul(t_[0:1, 0:1, 0:1], t_[0:1, 0:1, 0:1], 0.0)

                def emit_epi():
                    # per-sample epilogue: [P,2,64] APs, both i-halves at once.
                    # Per-partition-scalar multiplies ride ScalarE.
                    Sp_v = SQA[:, :, 0, :]
                    Qp_v = SQA[:, :, 1, :]
                    Sa_v = SQA[:, :, 2, :]

                    def et(tag):
                        return ep.tile([P, 2, DE], F32, tag=tag, name=tag)

                    tP_ = et("tP")
                    nc.scalar.mul(tP_[:], x0f[:], sc[(b, "npadP")][:])
                    tA2 = et("tA2")
                    nc.scalar.mul(tA2[:], xaf[:], sc[(b, "npadA")][:])
                    Spres = et("Spres")
                    TT(Spres[:], Sp_v, tP_[:], SUB)
                    Sabs = et("Sabs")
                    TT(Sabs[:], Sa_v, tA2[:], SUB)
                    tQ_ = et("tQ")
                    TT(tQ_[:], tP_[:], x0f[:], MUL)
                    Qpres = et("Qpres")
                    TT(Qpres[:], Qp_v, tQ_[:], SUB)
                    sall = et("sall")
                    TT(sall[:], Spres[:], Sabs[:], ADD)
                    m_dst = zS01[:, :, 0:64]  # strided 3D AP
                    nc.scalar.mul(m_dst, sall[:], sc[(b, "invCP")][:])
                    d_t = et("d")
                    TT(d_t[:], Spres[:], Sabs[:], SUB)
                    e_t = et("e")
                    TT(e_t[:], m_dst, d_t[:], MUL)
                    f_t = et("f")
                    TT(f_t[:], Qpres[:], e_t[:], SUB)
                    nc.scalar.mul(zS23[:, :, 64:128], f_t[:], sc[(b, "invCP")][:])

                def emit_out(ih):
                    r0 = b * N + ih * P
                    psz0 = psum.tile([P, P], F32, tag="psz0")
                    nc.tensor.transpose(out=psz0[:], in_=zS01[:, ih, :], identity=id_t[:])
                    psz1 = psum.tile([P, P], F32, tag="psz1")
                    nc.tensor.transpose(out=psz1[:], in_=zS23[:, ih, :], identity=id_t[:])
                    zT0 = outp.tile([P, P], F32, tag="zT0")
                    zT1 = outp.tile([P, P], F32, tag="zT1")
                    if last:
                        nc.vector.tensor_copy(out=zT0[:], in_=psz0[:])
                    else:
                        nc.scalar.copy(out=zT0[:], in_=psz0[:])
                    nc.scalar.copy(out=zT1[:], in_=psz1[:])

                    pso = psum.tile([P, DX], F32, tag="pso")
                    nc.tensor.matmul(pso[:], zT0[:], wt0[:], start=True, stop=False)
                    nc.tensor.matmul(pso[:], zT1[:], wt1[:], start=False, stop=False)
                    nc.tensor.matmul(pso[:], ones1[:], br_t[:], start=False, stop=True)
                    o_t = outp.tile([P, DX], F32, tag="o_t")
                    nc.scalar.copy(out=o_t[:], in_=pso[:])
                    nc.scalar.dma_start(out=out[r0 : r0 + P, :], in_=o_t[:])

                if not last:
                    for ih in range(2):
                        emit_load(ih)
                        emit_minmax(ih)
                        emit_sums(ih)
                        emit_fences(ih)
                    emit_epi()
                    for ih in range(2):
                        emit_out(ih)
                else:
                    # last sample: finish the sums (epilogue inputs) before the
                    # min/max trees so the output chain overlaps them
                    emit_load(0)
                    emit_minmax(0)
                    emit_sums(0)
                    emit_fences(0)
                    emit_load(1)
                    emit_sums(1)
                    emit_epi()
                    emit_out(0)
                    emit_minmax(1)
                    emit_fences(1)
                    emit_out(1)

    import json as _json

    _orig_to_json = nc.to_json_bytes

    def _patched_to_json():
        return _json.dumps(_hoist_excess_waits(_json.loads(_orig_to_json()))).encode()

    nc.to_json_bytes = _patched_to_json
    return nc


def kernel(E, e_mask2, W, b):
    E = np.asarray(E, dtype=np.float32)
    mask = np.asarray(e_mask2).reshape(BS, N).astype(bool)
    W = np.asarray(W, dtype=np.float32)
    bv = np.asarray(b, dtype=np.float32)

    pj = [np.nonzero(mask[s])[0] for s in range(BS)]
    aj = [np.nonzero(~mask[s])[0] for s in range(BS)]
    cPs = [len(x) for x in pj]
    cAs = [len(x) for x in aj]
    assert all(c > 0 for c in cPs), "a sample with zero present edges divides by zero"
    CA = max(1, max(cAs))
    CApad = 64 if CA <= 64 else 128
    assert CA <= 128

    perms = []
    for s in range(BS):
        pad_p = np.full(N - cPs[s], pj[s][0], dtype=np.int64)
        if cAs[s] > 0:
            tail = np.concatenate(
                [aj[s], np.full(CApad - cAs[s], aj[s][0], dtype=np.int64)]
            )
        else:
            tail = np.full(CApad, pj[s][0], dtype=np.int64)
        perms.append(np.concatenate([pj[s], pad_p, tail]))

    WT = np.ascontiguousarray(W.T)  # [FI, DX]
    ident = np.eye(P, dtype=np.float32)
    brow = np.ascontiguousarray(bv.reshape(1, DX))

    if CApad not in _NC_CACHE:
        _NC_CACHE[CApad] = build_program(CApad)
    nc = _NC_CACHE[CApad]

    in_maps = []
    for c in range(NCORES):
        egs = np.empty((BPC * N, N + CApad, DE), np.float16)
        scals = np.empty((BPC, P, 3), np.float32)
        for bl in range(BPC):
            s = c * BPC + bl
            egs[bl * N : (bl + 1) * N] = E[s][:, perms[s], :].astype(np.float16)
            npadA = (CApad - cAs[s]) if cAs[s] > 0 else CApad
            scals[bl, :] = (float(N - cPs[s]), float(npadA), 1.0 / cPs[s])
        in_maps.append(
            {"eg": egs, "wt": WT, "brow": brow, "ident": ident, "scal": scals}
        )

    trace = os.environ.get("NN_KERNEL_TRACE", "0") == "1"
    if trace:
        _enable_tracing()
    res = run_bass_kernel_spmd(
        nc, in_maps, list(range(NCORES)), trace=trace, tmpdir="/tmp/nn_kernel_trace"
    )
    LAST_RESULT["exec_time_ns"] = res.exec_time_ns
    LAST_RESULT["mean_exec_time_ns"] = res.mean_exec_time_ns
    LAST_RESULT["profile_json"] = res.profile_json

    out = np.concatenate(
        [res.results[c]["out"].reshape(BPC, N, DX) for c in range(NCORES)], axis=0
    )
    return out.astype(np.float32)


# revision 26
# speedup vs baseline: 1.1509x; 1.0116x over previous
"""Trainium2 Bass kernel for masked edge pooling + linear (nn_EtoX).

Reference computation (per sample b, node i, over neighbors j with mask[b, j]):
  m   = sum_j E[b,i,j,:] / count_b          (unmasked sum / masked count)
  mi  = min over present j of E[b,i,j,:]
  ma  = max over present j of E[b,i,j,:]
  std = sum_{present j} (E - m)^2 / count_b
  out = concat(m, mi, ma, std) @ W.T + bias

Strategy v2: data-parallel over batch (2 samples per core, 8 cores). The host
permutes each sample's j axis present-first (pads duplicate the first present
row) and appends the absent rows (padded to CApad with duplicates), then casts
to fp16. One contiguous DMA per 128-row i-block brings the merged
[128, 256+CApad, 64] fp16 slab in; all reductions are pairwise fp16
tensor_tensor trees on VectorE at 2x rate:
  - min/max trees over the 256 present-padded rows (pads are neutral)
  - sum tree (pad contribution subtracted exactly via npadP * x0)
  - ScalarE squares the slab; a second tree gives the present sum of squares
  - GpSimd reduces the absent block for the mean's unmasked-sum correction
The epilogue forms m and std in fp32 ([P,64] tiles, no parity split), packs
z = [m|mi] / [ma|std], and TensorE transposes + applies the 256x256 linear.
"""

import os

# Whole-tile dependency granularity: lets a 1-element ACT "fence" write
# supersede a DMA-landed tile's reader/writer dep set, keeping every DMA
# instruction within the hardware's 2-sync-wait budget.
os.environ.setdefault("BY_DEFAULT_DISABLE_SUBTILE_DEPS", "1")

import numpy as np

try:
    from concourse import bass, mybir, tile
    from concourse.bass_utils import run_bass_kernel_spmd
except ImportError:  # fall back to the container's repo checkout
    import sys

    sys.path.insert(0, "/opt/trn_rl_repo")
    from concourse import bass, mybir, tile
    from concourse.bass_utils import run_bass_kernel_spmd

BS, N, DE, DX = 16, 256, 64, 256
FI = 4 * DE
NCORES = 8
BPC = BS // NCORES  # samples per core
P = 128

F32 = mybir.dt.float32
F16 = mybir.dt.float16

LAST_RESULT = {}

_NC_CACHE = {}


def _enable_tracing():
    """Install the NTFF profile hook that the image's ``antenv`` lacks."""
    import contextlib
    import ctypes
    import sys
    import types

    try:
        import antenv.axon_hooks  # noqa: F401

        pass
    except ImportError:
        so_path = "/opt/axon/libaxon_pjrt.so"
        lib = ctypes.CDLL(so_path)
        if hasattr(lib, "axon_start_nrt_profile"):
            lib.axon_start_nrt_profile.argtypes = [
                ctypes.POINTER(ctypes.c_int64),
                ctypes.c_size_t,
            ]
            lib.axon_start_nrt_profile.restype = ctypes.c_int64
            lib.axon_stop_nrt_profile.argtypes = [ctypes.c_char_p]
            lib.axon_stop_nrt_profile.restype = ctypes.c_int64

            @contextlib.contextmanager
            def _hook(output_dir, device_ids):
                import jax

                jax.devices()
                if device_ids:
                    ids = (ctypes.c_int64 * len(device_ids))(*device_ids)
                    rc = lib.axon_start_nrt_profile(ids, len(device_ids))
                else:
                    rc = lib.axon_start_nrt_profile(None, 0)
                if rc != 0:
                    raise RuntimeError(f"axon_start_nrt_profile rc={rc}")
                try:
                    yield
                finally:
                    n = lib.axon_stop_nrt_profile(str(output_dir).encode())
                    print(f"profile: {n} file(s) written to {output_dir}")

            mod = types.ModuleType("antenv.axon_hooks")
            mod.get_axon_ntff_profile_hook = lambda: _hook
            mod.set_axon_ntff_profile_hook = lambda h: None
            import antenv

            sys.modules["antenv.axon_hooks"] = mod
            antenv.axon_hooks = mod

    from concourse import bass_utils as _bu

    _bu.upload_artifacts = lambda tmpdir: f"file://{tmpdir}"


def _hoist_excess_waits(bir: dict) -> dict:
    """Walrus (this build) rejects instructions whose embedded sync-wait list
    exceeds the ISA struct's slots. Hoist all but one wait into standalone
    single-wait EventSemaphore instructions placed immediately before the
    instruction on the same engine stream - semantically identical."""
    ctr = 0
    for fn in bir["functions"]:
        for blk in fn["blocks"]:
            new = []
            for ins in blk["instructions"]:
                si = ins.get("sync_info")
                if si:
                    waits = si.get("on_wait") or []
                    if len(waits) > 1:
                        for w in waits[:-1]:
                            ctr += 1
                            new.append(
                                {
                                    "debug": ins.get("debug", 0),
                                    "engine": ins["engine"],
                                    "ins": [],
                                    "outs": [],
                                    "name": f"hoistw-{ctr}",
                                    "opcode": "EventSemaphore",
                                    "sync_info": {"on_update": [], "on_wait": [w]},
                                }
                            )
                        si["on_wait"] = [waits[-1]]
                new.append(ins)
            blk["instructions"] = new
    return bir


def build_program(CApad: int) -> "bass.Bass":
    nc = bass.Bass()
    NI = BPC * N  # flattened (sample, i) rows
    W_ROW = N + CApad  # merged row length in j
    eg = nc.declare_dram_parameter("eg", [NI, W_ROW, DE], F16, isOutput=False)
    wt = nc.declare_dram_parameter("wt", [FI, DX], F32, isOutput=False)
    brow = nc.declare_dram_parameter("brow", [1, DX], F32, isOutput=False)
    ident = nc.declare_dram_parameter("ident", [P, P], F32, isOutput=False)
    scal = nc.declare_dram_parameter("scal", [BPC, P, 3], F32, isOutput=False)
    out = nc.declare_dram_parameter("out", [NI, DX], F32, isOutput=True)

    MIN = mybir.AluOpType.min
    MAX = mybir.AluOpType.max
    ADD = mybir.AluOpType.add
    SUB = mybir.AluOpType.subtract
    MUL = mybir.AluOpType.mult

    # SDMA-CCE accumulate DMAs crash this runtime (JaxRuntimeError INTERNAL on
    # both HBM->SBUF and SBUF->SBUF accum_op paths) - keep disabled.
    USE_CCE_S = os.environ.get("NN_CCE_S", "0") == "1"
    USE_CCE_Q = os.environ.get("NN_CCE_Q", "0") == "1"

    with tile.TileContext(nc) as tc:
        with (
            tc.tile_pool(name="singles", bufs=1) as singles,
            tc.tile_pool(name="main", bufs=2) as main,
            tc.tile_pool(name="sq", bufs=1) as sqp,
            tc.tile_pool(name="trees", bufs=1) as trees,
            tc.tile_pool(name="stats", bufs=2) as stats,
            tc.tile_pool(name="ep", bufs=1) as ep,
            tc.tile_pool(name="outp", bufs=2) as outp,
            tc.tile_pool(name="psum", bufs=2, space="PSUM") as psum,
        ):
            # singles are DMA'd on the scalar HWDGE ring AFTER the first
            # block's loads (they are not needed until ~70us in); this keeps
            # both rings' FIFO heads free for the first data tiles
            wt0 = singles.tile([P, DX], F32, tag="wt0")
            wt1 = singles.tile([P, DX], F32, tag="wt1")
            id_t = singles.tile([P, P], F32, tag="id")
            br_t = singles.tile([1, DX], F32, tag="br")
            ones1 = singles.tile([1, P], F32, tag="ones")
            nc.vector.memset(ones1[:], 1.0)
            sc = {}
            for b_ in range(BPC):
                for k, nm in enumerate(("npadP", "npadA", "invCP")):
                    sc[(b_, nm)] = singles.tile(
                        [P, 1], F32, tag=f"sc{b_}{k}", name=f"sc{b_}{k}"
                    )

            def emit_singles():
                nc.scalar.dma_start(out=wt0[:], in_=wt[0:P, :])
                nc.scalar.dma_start(out=wt1[:], in_=wt[P:FI, :])
                nc.scalar.dma_start(out=id_t[:], in_=ident[:, :])
                nc.scalar.dma_start(out=br_t[:], in_=brow[:, :])
                for b_ in range(BPC):
                    for k, nm in enumerate(("npadP", "npadA", "invCP")):
                        nc.scalar.dma_start(
                            out=sc[(b_, nm)][:], in_=scal[b_, :, k : k + 1]
                        )

            # shared DVE tree scratch (DVE-serial, bufs=1 is fine)
            tA = trees.tile([P, P, DE], F16, tag="treeA")
            tB = trees.tile([P, 64, DE], F16, tag="treeB")

            def tree_down(op, src, w0, dst_f32):
                """Pairwise-reduce src[:, 0:2*w0, :] (fp16) over j into the
                fp32 [P, 64] AP dst_f32, ping-ponging through tB/tA."""
                cur, nxt = src, tB
                w = w0
                while w >= 2:
                    nc.vector.tensor_tensor(
                        nxt[:, 0:w, :], cur[:, 0:w, :], cur[:, w : 2 * w, :], op
                    )
                    cur = nxt
                    nxt = tA if nxt is tB else tB
                    w //= 2
                nc.vector.tensor_tensor(
                    dst_f32,
                    cur[:, 0:1, :].rearrange("p a d -> p (a d)"),
                    cur[:, 1:2, :].rearrange("p a d -> p (a d)"),
                    op,
                )

            # packed-tail staging: [S-L3 | Q-L3 | abs-L1] as 3 groups of 32 rows
            pk0 = trees.tile([P, 3 * 32, DE], F16, tag="pk0")
            pk1 = trees.tile([P, 3 * 16, DE], F16, tag="pk1")

            for b in range(BPC):
                # per-sample stat tiles: index 'a' is the i-half (ih)
                zS01 = stats.tile([P, 2, P], F32, tag="z01")  # per ih: [m | mi]
                zS23 = stats.tile([P, 2, P], F32, tag="z23")  # per ih: [ma | std]
                SQA = stats.tile([P, 2, 3, DE], F32, tag="SQA")  # (S|Q|Sa) pad sums
                x0f = stats.tile([P, 2, DE], F32, tag="x0f")
                xaf = stats.tile([P, 2, DE], F32, tag="xaf")

                last = b == BPC - 1
                blk = {}

                def emit_load(ih):
                    # each 128-row j-half is split into two 64-row quarters
                    # issued on DIFFERENT HWDGE rings (sync + scalar) so the
                    # transfers run concurrently instead of FIFO-serialized
                    r0 = b * N + ih * P
                    qt = []
                    for h in range(4):
                        t_ = main.tile([P, 64, DE], F16, tag=f"mt{h}", name=f"mt{h}")
                        ring = nc.sync if h % 2 == 0 else nc.scalar
                        ring.dma_start(
                            out=t_[:], in_=eg[r0 : r0 + P, h * 64 : (h + 1) * 64, :]
                        )
                        qt.append(t_)
                    mta = main.tile([P, CApad, DE], F16, tag="mta")
                    nc.sync.dma_start(out=mta[:], in_=eg[r0 : r0 + P, N : N + CApad, :])
                    sq = []
                    for h in range(4):
                        s_ = sqp.tile([P, 64, DE], F16, tag=f"sq{h}", name=f"sq{h}")
                        nc.scalar.activation(
                            out=s_[:],
                            in_=qt[h][:],
                            func=mybir.ActivationFunctionType.Square,
                        )
                        sq.append(s_)
                    nc.scalar.copy(out=x0f[:, ih, :], in_=qt[0][:, 0, :])
                    nc.scalar.copy(out=xaf[:, ih, :], in_=mta[:, 0, :])
                    blk[ih] = (qt, mta, sq)

                TT = nc.vector.tensor_tensor

                def emit_minmax(ih):
                    qt, mta, sq = blk[ih]
                    TT(tA[:, 0:64, :], qt[0][:], qt[1][:], MIN)
                    TT(tA[:, 64:P, :], qt[2][:], qt[3][:], MIN)
                    tree_down(MIN, tA, 64, zS01[:, ih, 64:128])
                    TT(tA[:, 0:64, :], qt[0][:], qt[1][:], MAX)
                    TT(tA[:, 64:P, :], qt[2][:], qt[3][:], MAX)
                    tree_down(MAX, tA, 64, zS23[:, ih, 0:64])

                def emit_sums(ih):
                    qt, mta, sq = blk[ih]
                    TT(tA[:, 0:64, :], qt[0][:], qt[1][:], ADD)
                    TT(tA[:, 64:P, :], qt[2][:], qt[3][:], ADD)
                    TT(tB[:, 0:64, :], tA[:, 0:64, :], tA[:, 64:P, :], ADD)
                    TT(pk0[:, 0:32, :], tB[:, 0:32, :], tB[:, 32:64, :], ADD)
                    if CApad == 64:
                        TT(pk0[:, 64:96, :], mta[:, 0:32, :], mta[:, 32:64, :], ADD)
                    else:  # CApad == 128: one extra pre-level
                        TT(tB[:, 0:64, :], mta[:, 0:64, :], mta[:, 64:P, :], ADD)
                        TT(pk0[:, 64:96, :], tB[:, 0:32, :], tB[:, 32:64, :], ADD)
                    TT(tA[:, 0:64, :], sq[0][:], sq[1][:], ADD)
                    TT(tA[:, 64:P, :], sq[2][:], sq[3][:], ADD)
                    TT(tB[:, 0:64, :], tA[:, 0:64, :], tA[:, 64:P, :], ADD)
                    TT(pk0[:, 32:64, :], tB[:, 0:32, :], tB[:, 32:64, :], ADD)

                    v32 = pk0[:, 0:96, :].rearrange("p (g w) d -> p g w d", g=3)
                    v16 = pk1[:, 0:48, :].rearrange("p (g w) d -> p g w d", g=3)
                    TT(v16, v32[:, :, 0:16, :], v32[:, :, 16:32, :], ADD)
                    v8 = pk0[:, 0:24, :].rearrange("p (g w) d -> p g w d", g=3)
                    TT(v8, v16[:, :, 0:8, :], v16[:, :, 8:16, :], ADD)
                    v4 = pk1[:, 0:12, :].rearrange("p (g w) d -> p g w d", g=3)
                    TT(v4, v8[:, :, 0:4, :], v8[:, :, 4:8, :], ADD)
                    v2 = pk0[:, 0:6, :].rearrange("p (g w) d -> p g w d", g=3)
                    TT(v2, v4[:, :, 0:2, :], v4[:, :, 2:4, :], ADD)
                    TT(SQA[:, ih, :, :], v2[:, :, 0, :], v2[:, :, 1, :], ADD)

                def emit_fences(ih):
                    qt, mta, sq = blk[ih]
                    for t_ in (*qt, mta, *sq):
                        nc.scalar.mul(t_[0:1, 0:1, 0:1], t_[0:1, 0:1, 0:1], 0.0)

                def emit_epi():
                    # per-sample epilogue: [P,2,64] APs, both i-halves at once.
                    # Per-partition-scalar multiplies ride ScalarE.
                    Sp_v = SQA[:, :, 0, :]
                    Qp_v = SQA[:, :, 1, :]
                    Sa_v = SQA[:, :, 2, :]

                    def et(tag):
                        return ep.tile([P, 2, DE], F32, tag=tag, name=tag)

                    tP_ = et("tP")
                    nc.scalar.mul(tP_[:], x0f[:], sc[(b, "npadP")][:])
                    tA2 = et("tA2")
                    nc.scalar.mul(tA2[:], xaf[:], sc[(b, "npadA")][:])
                    Spres = et("Spres")
                    TT(Spres[:], Sp_v, tP_[:], SUB)
                    Sabs = et("Sabs")
                    TT(Sabs[:], Sa_v, tA2[:], SUB)
                    tQ_ = et("tQ")
                    TT(tQ_[:], tP_[:], x0f[:], MUL)
                    Qpres = et("Qpres")
                    TT(Qpres[:], Qp_v, tQ_[:], SUB)
                    sall = et("sall")
                    TT(sall[:], Spres[:], Sabs[:], ADD)
                    m_dst = zS01[:, :, 0:64]  # strided 3D AP
                    nc.scalar.mul(m_dst, sall[:], sc[(b, "invCP")][:])
                    d_t = et("d")
                    TT(d_t[:], Spres[:], Sabs[:], SUB)
                    e_t = et("e")
                    TT(e_t[:], m_dst, d_t[:], MUL)
                    f_t = et("f")
                    TT(f_t[:], Qpres[:], e_t[:], SUB)
                    nc.scalar.mul(zS23[:, :, 64:128], f_t[:], sc[(b, "invCP")][:])

                def emit_out(ih):
                    r0 = b * N + ih * P
                    psz0 = psum.tile([P, P], F32, tag="psz0")
                    nc.tensor.transpose(out=psz0[:], in_=zS01[:, ih, :], identity=id_t[:])
                    psz1 = psum.tile([P, P], F32, tag="psz1")
                    nc.tensor.transpose(out=psz1[:], in_=zS23[:, ih, :], identity=id_t[:])
                    zT0 = outp.tile([P, P], F32, tag="zT0")
                    zT1 = outp.tile([P, P], F32, tag="zT1")
                    if last:
                        nc.vector.tensor_copy(out=zT0[:], in_=psz0[:])
                    else:
                        nc.scalar.copy(out=zT0[:], in_=psz0[:])
                    nc.scalar.copy(out=zT1[:], in_=psz1[:])

                    pso = psum.tile([P, DX], F32, tag="pso")
                    nc.tensor.matmul(pso[:], zT0[:], wt0[:], start=True, stop=False)
                    nc.tensor.matmul(pso[:], zT1[:], wt1[:], start=False, stop=False)
                    nc.tensor.matmul(pso[:], ones1[:], br_t[:], start=False, stop=True)
                    o_t = outp.tile([P, DX], F32, tag="o_t")
                    nc.scalar.copy(out=o_t[:], in_=pso[:])
                    nc.scalar.dma_start(out=out[r0 : r0 + P, :], in_=o_t[:])

                if b == 0:
                    emit_load(0)
                    emit_singles()
                    emit_minmax(0)
                    emit_sums(0)
                    emit_fences(0)
                    emit_load(1)
                    emit_minmax(1)
                    emit_sums(1)
                    emit_fences(1)
                    emit_epi()
                    for ih in range(2):
                        emit_out(ih)
                elif not last:
                    for ih in range(2):
                        emit_load(ih)
                        emit_minmax(ih)
                        emit_sums(ih)
                        emit_fences(ih)
                    emit_epi()
                    for ih in range(2):
                        emit_out(ih)
                else:
                    # last sample: finish the sums (epilogue inputs) before the
                    # min/max trees so the output chain overlaps them
                    emit_load(0)
                    emit_minmax(0)
                    emit_sums(0)
                    emit_fences(0)
                    emit_load(1)
                    emit_sums(1)
                    emit_epi()
                    emit_out(0)
                    emit_minmax(1)
                    emit_fences(1)
                    emit_out(1)

    import json as _json

    _orig_to_json = nc.to_json_bytes

    def _patched_to_json():
        return _json.dumps(_hoist_excess_waits(_json.loads(_orig_to_json()))).encode()

    nc.to_json_bytes = _patched_to_json
    return nc


def kernel(E, e_mask2, W, b):
    E = np.asarray(E, dtype=np.float32)
    mask = np.asarray(e_mask2).reshape(BS, N).astype(bool)
    W = np.asarray(W, dtype=np.float32)
    bv = np.asarray(b, dtype=np.float32)

    pj = [np.nonzero(mask[s])[0] for s in range(BS)]
    aj = [np.nonzero(~mask[s])[0] for s in range(BS)]
    cPs = [len(x) for x in pj]
    cAs = [len(x) for x in aj]
    assert all(c > 0 for c in cPs), "a sample with zero present edges divides by zero"
    CA = max(1, max(cAs))
    CApad = 64 if CA <= 64 else 128
    assert CA <= 128

    perms = []
    for s in range(BS):
        pad_p = np.full(N - cPs[s], pj[s][0], dtype=np.int64)
        if cAs[s] > 0:
            tail = np.concatenate(
                [aj[s], np.full(CApad - cAs[s], aj[s][0], dtype=np.int64)]
            )
        else:
            tail = np.full(CApad, pj[s][0], dtype=np.int64)
        perms.append(np.concatenate([pj[s], pad_p, tail]))

    WT = np.ascontiguousarray(W.T)  # [FI, DX]
    ident = np.eye(P, dtype=np.float32)
    brow = np.ascontiguousarray(bv.reshape(1, DX))

    if CApad not in _NC_CACHE:
        _NC_CACHE[CApad] = build_program(CApad)
    nc = _NC_CACHE[CApad]

    in_maps = []
    for c in range(NCORES):
        egs = np.empty((BPC * N, N + CApad, DE), np.float16)
        scals = np.empty((BPC, P, 3), np.float32)
        for bl in range(BPC):
            s = c * BPC + bl
            egs[bl * N : (bl + 1) * N] = E[s][:, perms[s], :].astype(np.float16)
            npadA = (CApad - cAs[s]) if cAs[s] > 0 else CApad
            scals[bl, :] = (float(N - cPs[s]), float(npadA), 1.0 / cPs[s])
        in_maps.append(
            {"eg": egs, "wt": WT, "brow": brow, "ident": ident, "scal": scals}
        )

    trace = os.environ.get("NN_KERNEL_TRACE", "0") == "1"
    if trace:
        _enable_tracing()
    res = run_bass_kernel_spmd(
        nc, in_maps, list(range(NCORES)), trace=trace, tmpdir="/tmp/nn_kernel_trace"
    )
    LAST_RESULT["exec_time_ns"] = res.exec_time_ns
    LAST_RESULT["mean_exec_time_ns"] = res.mean_exec_time_ns
    LAST_RESULT["profile_json"] = res.profile_json

    out = np.concatenate(
        [res.results[c]["out"].reshape(BPC, N, DX) for c in range(NCORES)], axis=0
    )
    return out.astype(np.float32)
